# revision 1
# baseline (speedup 1.0000x reference)
"""LBQuantization Trainium2 Bass kernel (nn_LBQuantization_35021163331684).

Math: per channel (C = B*c = 96, HW = 512*512 pixels):
    mn, mx = min(x_ch), max(x_ch)
    pos_j  = rp_j * (mx - mn) + mn         (rp rows pre-sorted on host;
                                            pos is monotone in rp, so this
                                            equals sorting pos)
    out    = largest value in {mn, pos_1..pos_7} that is <= pixel
           = left_ends[searchsorted(pos, pixel, side='right')]

Device algorithm, per channel tile [128, 2048] (one channel = 262144 px):
    1. ONE fused min/max pass (custom DVE op, hand-edited uop program):
       the out stream is the running max of -x, drained through a
       stride-0 AP so only the final value (-min) lands in SBUF; the
       per-lane accumulator folds max(x) -> [128, 2] (-min, max) partials.
    2. gpsimd partition_all_reduce(max) -> global (-mn, mx), broadcast to
       all 128 partitions by the all-reduce itself.
    3. mn = -(-mn) and rng = mx + (-mn) on the Scalar engine (exact);
       thresholds via a fused affine custom DVE op: pos = rp*rng + mn
       (mul-then-add, matching the reference's rounding exactly).
    4. bucketize with 3 fused custom DVE ops (nested exact selects whose
       outputs are bit-exact copies of the threshold values; the first op
       fits 3 thresholds because its else-branch is the hardware constant
       -FLT_MAX rather than a runtime scalar):
         r1 = x>=t3 ? t3 : (x>=t2 ? t2 : (x>=t1 ? t1 : -FLT_MAX))
         r2 = x>=t5 ? t5 : (x>=t4 ? t4 : r1)
         r3 = x>=t7 ? t7 : (x>=t6 ? t6 : r2)
       then the mn-patch y = max(r3, mn) runs on the otherwise-idle
       GPSIMD engine (exact), off the DVE critical path.

Sharding: fully data-parallel over channels; 12 channels/core x 8 cores,
no collectives. Each core's 12 MiB shard is SBUF-resident (one HBM read
+ one write, ~70 us of DMA fully overlapped with ~107 us of DVE work).
The first channel is loaded/scanned in column chunks (DVE ramp) and the
last channel's patch is column-split (tail out-DMA overlap).
TimelineSim cost-model estimate: ~119 us/core.
"""

import sys

if "/opt/trn_rl_repo" not in sys.path:
    sys.path.insert(0, "/opt/trn_rl_repo")

import numpy as np

N_CORES = 8
B, CC, H, W = 32, 3, 512, 512
C_TOTAL = B * CC          # 96
C_PER = C_TOTAL // N_CORES  # 12
P = 128
FD = (H * W) // P         # 2048
R = 8                     # region_num

_CACHE: dict = {}


# --------------------------------------------------------------------------- #
# Custom DVE ops
# --------------------------------------------------------------------------- #
def _register_ops():
    """Register the 5 LBQ custom DVE ops into dve_ops.OPS (idempotent)."""
    if "ops" in _CACHE:
        return _CACHE["ops"]
    from concourse import dve_ops
    from concourse.dve_spec import (
        C0,
        C1,
        C3,
        AluOp,
        Spec,
        Src0,
        Src1,
        _spill_c3_to_src1,
        lower,
        scan,
        select,
    )
    from concourse.dve_uop import AluInp, DveOpSpec

    def mk(name, spec):
        if name in dve_ops._SUB_OPCODE_FOR_NAME:
            return next(op for op in dve_ops.OPS if op.name == name)
        row = dve_ops._CUSTOM_DVE_ROW_BASE + len(dve_ops.OPS)
        assert row < 0x20, "custom DVE opcode rows exhausted"
        dve_ops._SUB_OPCODE_FOR_NAME[name] = row
        shas = {}
        for ver in ("v3", "v4"):
            try:
                shas[ver] = DveOpSpec(
                    name=name,
                    opcode=row,
                    uops=lower(spec, ver=ver),
                    rd1_en=dve_ops.has_src1(spec),
                ).sha(ver)
            except ValueError:
                pass  # ver not supported; we only need v3 (TRN2)
        assert "v3" in shas, f"{name}: v3 lowering failed"
        op = dve_ops.DveOp(name, spec, subdim=False, uops_sha=shas)
        dve_ops.OPS.append(op)
        dve_ops.CUSTOM_DVE_SPECS[name] = spec
        return op

    def _sel2_ref(in0, in1, c0, c1, c2):
        return np.where(in0 >= c1, c1, np.where(in0 >= c0, c0, in1)).astype(
            np.float32
        )

    # r = x>=t_c ? t_c : (x>=t_b ? t_b : (x>=t_a ? t_a : -FLT_MAX))
    # [t_c via C3 -> in1 [P,1]; the -FLT_MAX else is patched to mn by a
    #  gpsimd tensor_scalar_max after the chain]
    from concourse.dve_spec import MaxNeg

    sel3 = mk(
        "LBQ_SEL3N",
        Spec(
            body=_spill_c3_to_src1(
                select(
                    Src0 >= C3,
                    C3,
                    select(Src0 >= C1, C1, select(Src0 >= C0, C0, MaxNeg)),
                )
            ),
            reference=lambda in0, in1, c0, c1, c2: np.where(
                in0 >= in1, in1,
                np.where(
                    in0 >= c1, c1,
                    np.where(in0 >= c0, c0, np.float32(-3.4028235e38)),
                ),
            ).astype(np.float32),
        ),
    )
    # r = x>=t_b ? t_b : (x>=t_a ? t_a : carry)   [carry via Src1 [P,N]]
    sel2c = mk(
        "LBQ_SEL2C",
        Spec(
            body=select(Src0 >= C1, C1, select(Src0 >= C0, C0, Src1)),
            reference=_sel2_ref,
        ),
    )
    # r = x>=t ? t : carry
    sel1c = mk(
        "LBQ_SEL1C",
        Spec(
            body=select(Src0 >= C0, C0, Src1),
            reference=lambda in0, in1, c0, c1, c2: np.where(
                in0 >= c0, c0, in1
            ).astype(np.float32),
        ),
    )
    # pos = rp*rng + mn  (per-partition scalars)
    affine = mk(
        "LBQ_AFFINE",
        Spec(
            body=Src0 * C0 + C1,
            reference=lambda in0, in1, c0, c1, c2: (
                in0.astype(np.float32) * c0 + c1
            ).astype(np.float32),
        ),
    )

    # Single-pass dual min/max: body = running max of (-x) (written through
    # a stride-0 out AP so only the final value, -min, lands), accum = MAX.
    # The DSL accum folds the body root (the -x scan), which is not what we
    # want -- so after lowering, rewire the accum stage's B input from
    # PREV_ALU_OUT (scan result) to the delay lane carrying raw Src0 (the
    # lane is pass-through-wired across all stages by the lowerer).
    # The edited program is pre-seeded into dve_ops._COMPILE_CACHE, which
    # DveOp.compile() consults before re-lowering.
    def _minmax_ref(in0, in1, c0, c1, c2):
        x = in0.astype(np.float32)
        negmins = np.maximum.accumulate(np.maximum(-x, np.float32(c0)), axis=-1)
        mx = x.reshape(x.shape[0], -1).max(axis=-1, keepdims=True)
        return negmins, np.maximum(mx, np.float32(-3.4028235e38))

    from concourse.dve_spec import Zero

    mm_name = "LBQ_MINMAX"
    if mm_name not in dve_ops._SUB_OPCODE_FOR_NAME:
        # out stream = running max of (-x) = -(running min); accum = max(x).
        mm_spec = Spec(
            body=scan(AluOp.MAX, Zero - Src0, init=C0),
            accum=AluOp.MAX,
            reference=_minmax_ref,
        )
        row = dve_ops._CUSTOM_DVE_ROW_BASE + len(dve_ops.OPS)
        assert row < 0x20
        dve_ops._SUB_OPCODE_FOR_NAME[mm_name] = row
        uops = lower(mm_spec, ver="v3")
        steady = uops[-1]
        # accum stage: stage after the scan combine; rewire its B input from
        # PREV_ALU_OUT (the scan value) to the delay lane carrying raw Src0.
        acc_st = None
        src0_lane = None
        for st, dp in enumerate(steady.datapath_config):
            if int(dp.alu_out_a_enable):  # first accum-tail stage = accum
                assert dp.op == AluOp.MAX and dp.alu_src1 == AluInp.PREV_ALU_OUT
                acc_st = st
                break
        for lane_idx in range(1, 7):
            if int(steady.inp_enable[lane_idx]) and steady.inp[lane_idx].name == "SRC_0":
                src0_lane = lane_idx - 1  # input lane N feeds delay_{N-1}
                break
        assert acc_st is not None and src0_lane is not None, (acc_st, src0_lane)
        steady.datapath_config[acc_st].alu_src1 = AluInp(
            int(AluInp.PREV_DELAY_0) + src0_lane
        )
        compiled = DveOpSpec(name=mm_name, opcode=row, uops=uops, rd1_en=False)
        minmax = dve_ops.DveOp(
            mm_name,
            mm_spec,
            subdim=False,
            uops_sha={"v3": compiled.sha("v3")},
        )
        dve_ops._COMPILE_CACHE[(mm_name, "v3")] = compiled
        dve_ops.OPS.append(minmax)
        dve_ops.CUSTOM_DVE_SPECS[mm_name] = mm_spec
    else:
        minmax = next(op for op in dve_ops.OPS if op.name == mm_name)

    _CACHE["ops"] = (sel3, sel2c, sel1c, affine, minmax)
    return _CACHE["ops"]


# --------------------------------------------------------------------------- #
# Bass module (SPMD: same program on all 8 cores, different data)
# --------------------------------------------------------------------------- #
def _build_module():
    if "nc" in _CACHE:
        return _CACHE["nc"]
    import concourse.bacc as bacc
    import concourse.bass as bass
    import concourse.bass_isa as bass_isa
    import concourse.tile as tile
    from concourse import mybir

    SEL3, SEL2C, SEL1C, AFFINE, MINMAX = _register_ops()
    f32 = mybir.dt.float32
    FLT_MAX = 3.4028234663852886e38

    nc = bacc.Bacc("TRN2", target_bir_lowering=False, name="lbq")
    x_d = nc.dram_tensor("x", [C_PER, P, FD], f32, kind="ExternalInput")
    rp_d = nc.dram_tensor("rp", [C_PER, R - 1], f32, kind="ExternalInput")
    y_d = nc.dram_tensor("y", [C_PER, P, FD], f32, kind="ExternalOutput")

    with tile.TileContext(nc) as tc:
        with (
            tc.tile_pool(name="xp", bufs=1) as xp,
            tc.tile_pool(name="wp", bufs=2) as wp,
            tc.tile_pool(name="sp", bufs=1) as sp,
            tc.tile_pool(name="op", bufs=3) as op_,
        ):
            # rp [12,7] DRAM -> one SBUF row -> gpsimd broadcast to [128, 84]
            # (a single 336 B DMA + idle-GPSIMD broadcast keeps the DMA
            # queues free for channel 0's data, which gates the DVE ramp)
            rp_b = sp.tile([P, C_PER, R - 1], f32, tag="rp_b")
            rp_row = sp.tile([1, C_PER * (R - 1)], f32, tag="rp_row")
            rp_ap = rp_d[:, :]
            nc.sync.dma_start(
                out=rp_row,
                in_=bass.AP(
                    tensor=rp_ap.tensor,
                    offset=rp_ap.offset,
                    ap=[[0, 1], [1, C_PER * (R - 1)]],
                ),
            )
            nc.gpsimd.partition_broadcast(
                rp_b.rearrange("p c r -> p (c r)"), rp_row, channels=P
            )
            def minmax_sink(dst_negmin, dst_max, src, fd):
                # out stream = running max of (-x) through a stride-0 free AP
                # so only the last (= -min) lands; accum_out = per-part max.
                sink = bass.AP(
                    tensor=dst_negmin.tensor,
                    offset=dst_negmin.offset,
                    ap=[list(dst_negmin.ap[0]), [0, fd]],
                )
                nc.vector._custom_dve(
                    MINMAX, out=sink, in0=src,
                    s0=-FLT_MAX, accum_out=dst_max,
                )

            for c in range(C_PER):
                xt = xp.tile([P, FD], f32, tag=f"x{c}")
                pm = sp.tile([P, 2], f32, tag=f"pm{c}")
                if c == 0:
                    # channel 0 gates the DVE pipeline ramp: load + scan it
                    # in 4 column chunks so the first compute starts ~3us
                    # earlier, then fold the chunk partials.
                    n_ck = 4
                    ck = FD // n_ck
                    pm8 = sp.tile([P, 2, n_ck], f32, tag="pm_ck")
                    for i in range(n_ck):
                        sl = slice(i * ck, (i + 1) * ck)
                        nc.sync.dma_start(out=xt[:, sl], in_=x_d[c][:, sl])
                        minmax_sink(
                            pm8[:, 0, i : i + 1], pm8[:, 1, i : i + 1],
                            xt[:, sl], ck,
                        )
                    nc.vector.tensor_reduce(
                        out=pm[:, 0:1], in_=pm8[:, 0, :],
                        axis=mybir.AxisListType.X, op=mybir.AluOpType.max,
                    )
                    nc.vector.tensor_reduce(
                        out=pm[:, 1:2], in_=pm8[:, 1, :],
                        axis=mybir.AxisListType.X, op=mybir.AluOpType.max,
                    )
                else:
                    nc.sync.dma_start(out=xt, in_=x_d[c])
                    minmax_sink(pm[:, 0:1], pm[:, 1:2], xt, FD)

                # cross-partition: ar[:,0] = -mn, ar[:,1] = mx (all partitions)
                ar = sp.tile([P, 2], f32, tag=f"ar{c}")
                nc.gpsimd.partition_all_reduce(
                    ar, pm, P, bass_isa.ReduceOp.max
                )

                # thr[:,0]=mn, thr[:,1..7]=pos_j = rp_j*rng + mn.
                # mn = -(-mn) and rng = mx + (-mn) run on the idle Scalar
                # engine (exact: Copy is a scale-multiply datapath, Identity
                # adds the [P,1] bias) to keep DVE streaming.
                thr = sp.tile([P, R], f32, tag=f"thr{c}")
                rng = sp.tile([P, 1], f32, tag=f"rng{c}")
                nc.scalar.activation(
                    out=thr[:, 0:1], in_=ar[:, 0:1],
                    func=mybir.ActivationFunctionType.Copy, scale=-1.0,
                )
                nc.scalar.activation(
                    out=rng, in_=ar[:, 1:2],
                    func=mybir.ActivationFunctionType.Identity,
                    bias=ar[:, 0:1], scale=1.0,
                )
                # pos = rp*rng + mn on DVE (custom op): the Scalar engine's
                # Identity scale-multiply is NOT correctly rounded (measured:
                # 1-ulp threshold shifts misbucket a handful of pixels), so
                # this must stay on the exact DVE datapath.
                nc.vector._custom_dve(
                    AFFINE, out=thr[:, 1:R], in0=rp_b[:, c, :],
                    s0=rng[:, 0:1], s1=thr[:, 0:1],
                )

                # bucketize: 3 chained fused selects on DVE ({t1,t2,t3} with
                # a -FLT_MAX else, then {t4,t5}, {t6,t7}), and the mn-patch
                # max(r, mn) on the otherwise-idle GPSIMD engine (exact).
                c1t = wp.tile([P, FD], f32, tag="carry1")
                c2t = wp.tile([P, FD], f32, tag="carry2")
                nc.vector._custom_dve(
                    SEL3, out=c1t, in0=xt, in1=thr[:, 3:4],
                    s0=thr[:, 1:2], s1=thr[:, 2:3],
                )
                nc.vector._custom_dve(
                    SEL2C, out=c2t, in0=xt, in1=c1t,
                    s0=thr[:, 4:5], s1=thr[:, 5:6],
                )
                nc.vector._custom_dve(
                    SEL2C, out=c1t, in0=xt, in1=c2t,
                    s0=thr[:, 6:7], s1=thr[:, 7:8],
                )
                ot = op_.tile([P, FD], f32, tag="out")
                if c == C_PER - 1:
                    # last channel gates the kernel tail: split the patch
                    # column-wise so the out-DMA overlaps it.
                    h = FD // 2
                    for s0_, s1_ in ((0, h), (h, FD)):
                        nc.gpsimd.tensor_scalar_max(
                            ot[:, s0_:s1_], c1t[:, s0_:s1_], thr[:, 0:1]
                        )
                        nc.sync.dma_start(
                            out=y_d[c][:, s0_:s1_], in_=ot[:, s0_:s1_]
                        )
                else:
                    nc.gpsimd.tensor_scalar_max(ot, c1t, thr[:, 0:1])
                    nc.sync.dma_start(out=y_d[c], in_=ot)

    nc.compile()
    _CACHE["nc"] = nc
    return nc


# --------------------------------------------------------------------------- #
# Host entry point
# --------------------------------------------------------------------------- #
def kernel(x, region_percentiles, _trace=False):
    x = np.asarray(x)
    in_dtype = x.dtype
    xs = np.ascontiguousarray(x, dtype=np.float32).reshape(
        N_CORES, C_PER, P, FD
    )
    rp = np.sort(
        np.ascontiguousarray(region_percentiles, dtype=np.float32), axis=1
    ).reshape(N_CORES, C_PER, R - 1)

    nc = _build_module()
    from concourse.bass_utils import run_bass_kernel_spmd

    in_maps = [{"x": xs[i], "rp": np.ascontiguousarray(rp[i])} for i in range(N_CORES)]
    res = run_bass_kernel_spmd(
        nc, in_maps, core_ids=list(range(N_CORES)), trace=_trace
    )
    _CACHE["last_result"] = res
    y = np.stack([res.results[i]["y"] for i in range(N_CORES)])
    return y.reshape(B, CC, H, W).astype(in_dtype)

